# revision 61
# baseline (speedup 1.0000x reference)
"""MultiHeadGeometryAttention Trainium2 kernel (v3).

Sharding: 8 cores = (B=2) x (N=2048 split into 4 query chunks of 512).
Each core computes the NxN geometry bias + side gate once for its 512
queries (shared by all 4 heads), then all-head attention in transposed
layout S^T[j, i] so the PV matmul contracts over keys on partitions.

Key structure (156 us baseline -> this kernel):
  - exp(-|crv_i - crv_j|) * normal_sim folded into the geometry matmul
    as a rank-64 separable expansion (PE contraction rows are free).
  - aniso term = (A2'/50) dp^2 - d2 with sqrt(A2'/50) folded into the
    d_par rhs; bias accumulates in ONE PSUM bank (ACT Square writes
    dp'^2, the -d2 / expansion matmuls accumulate on top); one eviction
    per tile, folded into scores via a 50*I fp16 identity matmul.
  - Gate sigmoid computed as (1 + tanh(-8*(|lat|-E)))/2 so every ACT
    function (Exp/Square/Abs/Tanh/Copy) lives in ONE table set -> no
    table reloads -> geometry and attention fully interleave per key
    tile inside 8 PSUM banks: geometry pa+pb (pa reused for lat),
    score half-groups 2x2, PV accumulators 2 (4 heads at partition
    bases 0/64).
  - Scores: one batched scalar_tensor_tensor per 2-head half (gate
    broadcast over heads), one batched exp per (jt, 4 heads) with bias
    -2 so e^s stays in fp16 range (cancels in softmax), PV pipelined
    one jt behind so PE never waits on the exp.
  - All tensors ship fp16 (hi/lo split hi parts fp16-exact) in few
    packed DMA blobs; fp16 matmuls run at full PE rate.
"""

import math

import numpy as np

import concourse.bass as bass  # noqa: F401
import concourse.mybir as mybir
import concourse.tile as tile
from concourse import bacc
from concourse.bass_utils import run_bass_kernel_spmd

# problem constants (fixed by the nn.Module config)
ALPHA0 = 1.0
BETA0 = 4.0
GAMMA = 0.5
SIGMA = 0.2
W_MIN, W_MAX = 0.05, 0.3
B, N, D, H = 2, 2048, 128, 4
HD = D // H  # 32
CH = 512  # query rows per core
NCORES = 8
NJT = N // 128  # 16 key tiles
NSUB = CH // 128  # 4 query subtiles

HALF_W = 0.5 * (W_MIN + W_MAX)  # 0.175
GATE_INV_SCALE = 1.0 / (0.25 * (W_MAX - W_MIN))  # 16.0
SC = 1.0 / math.sqrt(HD)
FOLD = 50.0  # geom tile carries bias/FOLD; folded back via FOLD*I matmul

R_EXP = 37  # rank of the exp(-|ci-cj|) separable expansion (geo rows 17..127)

LAG = 4  # attention trails geometry by this many key tiles
# geometry eviction engine per jt: True -> ACT, False -> DVE
GEOM_ACT = [False] * 16

# blob16 column offsets (keys are rotated per-core so xtc = xt[:, 0:CH])
C_I50, C_WV, C_WK, C_WO, C_ONES, C_E4 = 0, 128, 256, 768, 1280, 1408
B16C = 1536
# blob32: rhsm0 0-512, bq 512-516, bk 516-520, bv 520-524, ob row0 524-652
B32C = 652

F32 = mybir.dt.float32
F32R = mybir.dt.float32r
F16 = mybir.dt.float16
AF = mybir.ActivationFunctionType
ALU = mybir.AluOpType

_cache = {}


def _build_program(has_bk=False, has_bv=False):
    nc = bacc.Bacc(None)

    blob16 = nc.dram_tensor("blob16", [128, B16C], F16, kind="ExternalInput")
    blob32 = nc.dram_tensor("blob32", [128, B32C], F32, kind="ExternalInput")
    xt = nc.dram_tensor("xt", [D, N], F16, kind="ExternalInput")
    geo = nc.dram_tensor("geo", [128, N], F16, kind="ExternalInput")
    rhs = nc.dram_tensor("rhs", [128, 6, CH], F16, kind="ExternalInput")
    out = nc.dram_tensor("out", [128, NSUB, D], F32, kind="ExternalOutput")

    with tile.TileContext(nc) as tc, nc.allow_low_precision(
        reason="fp16 operands and f32r rounding are intentional"
    ):
        with (
            tc.tile_pool(name="const", bufs=1) as const,
            tc.tile_pool(name="tmp", bufs=1) as tmp,
            tc.tile_pool(name="s2p", bufs=4) as s2p,
            tc.tile_pool(name="ep", bufs=4) as ep,
            tc.tile_pool(name="gprep", bufs=2) as gprep,
            tc.tile_pool(name="latp", bufs=2) as latp,
        ):
            geo_sb = const.tile([128, N], F16, tag="geo", name="geo_sb")
            rhs_sb = const.tile([128, 6, CH], F16, tag="rhs", name="rhs_sb")
            b16 = const.tile([128, B16C], F16, tag="b16", name="b16")
            xt_sb = const.tile([D, N], F16, tag="xt", name="xt_sb")
            b32 = const.tile([128, B32C], F32, tag="b32", name="b32")
            nc.sync.dma_start(geo_sb[:, 0 : N // 2], geo[:, 0 : N // 2])
            nc.sync.dma_start(rhs_sb[:, 4:6, :], rhs[:, 4:6, :])
            nc.sync.dma_start(rhs_sb[:, 0:4, :], rhs[:, 0:4, :])
            nc.sync.dma_start(geo_sb[:, N // 2 : N], geo[:, N // 2 : N])
            nc.sync.dma_start(b16, blob16[...])
            nc.sync.dma_start(xt_sb[:, 0 : N // 2], xt[:, 0 : N // 2])
            nc.sync.dma_start(xt_sb[:, N // 2 : N], xt[:, N // 2 : N])
            nc.sync.dma_start(b32, blob32[...])

            xtc_sb = xt_sb[:, 0:CH]
            i50_sb = b16[:, C_I50 : C_I50 + D]
            wv_sb = b16[:, C_WV : C_WV + D]
            ones_sb = b16[0:1, C_ONES : C_ONES + D]

            ob_sb = const.tile([1, D], F16, tag="ob", name="ob_sb")
            nc.gpsimd.tensor_copy(ob_sb, b32[0:1, 524 : 524 + D])

            nb2 = const.tile([128, 1], F32, tag="nb2", name="nb2")
            nc.gpsimd.memset(nb2, -2.0)

            qk4 = const.tile([D, H, CH], F16, tag="qk4", name="qk4")
            V = const.tile([128, NJT, H, HD + 1], F16, tag="V", name="V")
            E16 = const.tile([128, CH], F16, tag="E16", name="E16")
            geom16 = const.tile([128, NJT, CH], F16, tag="geom16", name="geom16")
            gate16 = const.tile([128, NJT, CH], F16, tag="gate16", name="gate16")
            headcat = const.tile([HD, H, CH], F16, tag="headcat", name="headcat")

            nc.gpsimd.memset(V[:, :, :, HD : HD + 1], 1.0)

            # ---- main loop: geometry and attention interleaved ----
            with (
                tc.tile_pool(name="a_pv", bufs=1, space="PSUM") as a_pv,
            ):
                pv0 = a_pv.tile([128, CH], F32, tag="pv0", name="pv0")
                pv1 = a_pv.tile([128, CH], F32, tag="pv1", name="pv1")
                pvb = [pv0, pv1]
                prev = [None]

                with (
                    tc.tile_pool(name="g_pa", bufs=1, space="PSUM") as g_pa,
                    tc.tile_pool(name="g_pb", bufs=1, space="PSUM") as g_pb,
                    tc.tile_pool(name="a_sg", bufs=2, space="PSUM") as a_sg,
                ):
                    gpre = [None]

                    # qk4_h = M_h^T @ x_chunk with M_h = SC * Wq_h Wk_h^T
                    # host-folded (kills the q->qT->k chain on device)
                    for h in range(2):
                        sgk = a_sg.tile([128, 2, CH], F32, tag="sg",
                                        name="sgk")
                        for hh in range(2):
                            nc.tensor.matmul(
                                sgk[:, hh, :],
                                lhsT=b16[:, C_WK + (2 * h + hh) * D
                                         : C_WK + (2 * h + hh + 1) * D],
                                rhs=xtc_sb,
                                start=True,
                                stop=True,
                            )
                        nc.scalar.copy(qk4[:, 2 * h : 2 * h + 2, :], sgk)
                    sge = a_sg.tile([128, 2, CH], F32, tag="sg", name="sge")
                    nc.tensor.matmul(
                        sge[:, 0, :], lhsT=ones_sb, rhs=rhs_sb[0:1, 3, :],
                        start=True, stop=True,
                    )
                    nc.scalar.copy(E16, sge[:, 0, :])

                    def emit_g(jt):
                        j0 = jt * 128
                        lhs = geo_sb[:, j0 : j0 + 128]
                        pa1 = g_pa.tile([128, CH], F32, tag="pa", name="pa1")
                        nc.tensor.matmul(pa1, lhsT=lhs, rhs=rhs_sb[:, 4, :],
                                         start=True, stop=False)
                        nc.tensor.matmul(pa1, lhsT=lhs, rhs=rhs_sb[:, 5, :],
                                         start=False, stop=True,
                                         skip_group_check=True)
                        pb = g_pb.tile([128, CH], F32, tag="pb", name="pb")
                        nc.scalar.activation(pb, pa1, AF.Square)
                        pa2 = g_pa.tile([128, CH], F32, tag="pa", name="pa2")
                        nc.tensor.matmul(pa2, lhsT=lhs, rhs=rhs_sb[:, 2, :],
                                         start=True, stop=True)
                        nc.tensor.matmul(
                            pb, lhsT=lhs, rhs=rhs_sb[:, 0, :], start=False,
                            stop=False, skip_group_check=True,
                        )
                        nc.tensor.matmul(
                            pb, lhsT=lhs, rhs=rhs_sb[:, 1, :], start=False,
                            stop=True, skip_group_check=True,
                        )
                        if GEOM_ACT[jt]:
                            nc.scalar.copy(geom16[:, jt, :], pb)
                        else:
                            nc.vector.tensor_copy(geom16[:, jt, :], pb)
                        lata = latp.tile([128, CH], F16, tag="lata",
                                         name="lata")
                        nc.scalar.activation(lata, pa2, AF.Abs)
                        if jt % 2 == 0:
                            gpre[0] = gprep.tile([128, 2, CH], F16, tag="gp",
                                                 name="gpre")
                        nc.vector.tensor_tensor(
                            gpre[0][:, jt % 2, :], lata, E16, ALU.subtract
                        )
                        if jt % 2 == 1:
                            g = jt // 2
                            tgrp = latp.tile([128, 2, CH], F16, tag="tgrp",
                                             name="tgrp")
                            nc.scalar.activation(
                                tgrp, gpre[0], AF.Tanh,
                                scale=-0.5 * GATE_INV_SCALE,
                            )
                            # gate = 0.5 * (tanh + 1)
                            nc.vector.tensor_scalar(
                                gate16[:, g * 2 : g * 2 + 2, :], tgrp, 1.0,
                                0.5, ALU.add, ALU.mult,
                            )

                    def emit_a(jt):
                        s2 = s2p.tile([128, H, CH], F16, tag="s2", name="s2")
                        e = ep.tile([128, H, CH], F16, tag="e", name="e")
                        for half in range(2):
                            sg = a_sg.tile([128, 2, CH], F32, tag="sg",
                                           name="sg")
                            for hh in range(2):
                                h = 2 * half + hh
                                nc.tensor.matmul(
                                    sg[:, hh, :],
                                    lhsT=xt_sb[:, jt * 128 : (jt + 1) * 128],
                                    rhs=qk4[:, h, :],
                                    start=True,
                                    stop=False,
                                )
                                nc.tensor.matmul(
                                    sg[:, hh, :],
                                    lhsT=i50_sb,
                                    rhs=geom16[:, jt, :],
                                    start=False,
                                    stop=True,
                                    skip_group_check=True,
                                )
                            nc.vector.scalar_tensor_tensor(
                                s2[:, 2 * half : 2 * half + 2, :],
                                sg,
                                0.0,
                                gate16[:, jt : jt + 1, :].to_broadcast(
                                    [128, 2, CH]
                                ),
                                ALU.bypass,
                                ALU.mult,
                            )
                            nc.scalar.activation(
                                e[:, 2 * half : 2 * half + 2, :],
                                s2[:, 2 * half : 2 * half + 2, :],
                                AF.Exp, bias=nb2,
                            )
                        if prev[0] is not None:
                            emit_pv(*prev[0])
                        prev[0] = (jt, e)

                    def emit_pv(jt, e):
                        for h in range(H):
                            b0 = (h % 2) * 64
                            nc.tensor.matmul(
                                pvb[h // 2][b0 : b0 + HD + 1, :],
                                lhsT=V[:, jt, h, :],
                                rhs=e[:, h, :],
                                start=(jt == 0),
                                stop=(jt == NJT - 1),
                                skip_group_check=True,
                            )

                    def emit_v(vj):
                        sg = a_sg.tile([128, 2, CH], F32, tag="sg", name="sgv")
                        nc.tensor.matmul(
                            sg[:, 0, 0:D],
                            lhsT=xt_sb[:, vj * 128 : (vj + 1) * 128],
                            rhs=wv_sb,
                            start=True,
                            stop=True,
                        )
                        nc.vector.tensor_copy(
                            out=V[:, vj, :, 0:HD],
                            in_=sg[:, 0, 0:D].rearrange("p (h d) -> p h d",
                                                        h=H),
                        )

                    vper = (NJT + LAG - 1) // LAG  # V-projections per G step
                    for jt in range(NJT):
                        emit_g(jt)
                        if jt < LAG:
                            for vj in range(jt * vper,
                                            min((jt + 1) * vper, NJT)):
                                emit_v(vj)
                        else:
                            emit_a(jt - LAG)
                    for jt in range(NJT - LAG, NJT):
                        emit_a(jt)
                    emit_pv(*prev[0])

                # ---- finish: normalize, concat heads, project ----
                with tc.tile_pool(name="f_ps", bufs=2, space="PSUM") as f_ps:
                    recips = []
                    for h in range(H):
                        b0 = (h % 2) * 64
                        recip = tmp.tile([1, CH], F16, tag="recip",
                                         name="recip", bufs=4)
                        nc.vector.reciprocal(
                            recip, pvb[h // 2][b0 + HD : b0 + HD + 1, :]
                        )
                        recips.append(recip)
                    bc16s = []
                    for pair in range(2):
                        bc_ps = f_ps.tile([2 * HD, CH], F32, tag="bc",
                                          name="bc_ps")
                        for hh in range(2):
                            nc.tensor.matmul(
                                bc_ps[hh * HD : (hh + 1) * HD, :],
                                lhsT=ones_sb[0:1, 0:HD],
                                rhs=recips[2 * pair + hh],
                                start=True,
                                stop=True,
                            )
                        bc16 = tmp.tile([2 * HD, CH], F16, tag="bc16",
                                        name="bc16", bufs=2)
                        nc.scalar.copy(bc16, bc_ps)
                        bc16s.append(bc16)
                    for h in range(H):
                        b0 = (h % 2) * 64
                        nc.vector.scalar_tensor_tensor(
                            headcat[:, h, :],
                            pvb[h // 2][b0 : b0 + HD, :],
                            0.0,
                            bc16s[h // 2][(h % 2) * HD : (h % 2 + 1) * HD, :],
                            ALU.bypass,
                            ALU.mult,
                        )
                        if has_bv:
                            nc.scalar.activation(
                                headcat[:, h, :], headcat[:, h, :],
                                AF.Identity,
                                bias=b32[0:HD, 520 + h : 521 + h],
                            )

                    f_all = tmp.tile([128, NSUB, D], F32, tag="fall",
                                     name="f_all")
                    for s in range(NSUB):
                        fps = f_ps.tile([128, D], F32, tag="f", name="fps")
                        for h in range(H):
                            nc.tensor.matmul(
                                fps,
                                lhsT=headcat[:, h, s * 128 : (s + 1) * 128],
                                rhs=b16[0:HD, C_WO + h * D : C_WO + (h + 1) * D],
                                start=(h == 0),
                                stop=False,
                            )
                        nc.tensor.matmul(
                            fps, lhsT=ones_sb, rhs=ob_sb, start=False,
                            stop=True,
                        )
                        if s % 2:
                            nc.scalar.copy(f_all[:, s, :], fps)
                        else:
                            nc.vector.tensor_copy(f_all[:, s, :], fps)
                    nc.sync.dma_start(out[...], f_all)

    nc.finalize()
    return nc


def _split_hi_lo(v):
    """Split fp32 array into an fp16-exact hi part and the fp32 residual."""
    v = np.asarray(v, np.float32)
    hi = v.astype(np.float16).astype(np.float32)
    lo = (v.astype(np.float64) - hi).astype(np.float32)
    return hi, lo


_exp_basis = None


def _get_exp_basis():
    """Separable rank-R_EXP expansion of exp(-|a-b|) on [0,1]^2."""
    global _exp_basis
    if _exp_basis is None:
        g = np.linspace(0.0, 1.0, 2048)
        K = np.exp(-np.abs(g[:, None] - g[None, :]))
        U, s, Vt = np.linalg.svd(K)
        r = R_EXP
        FI = U[:, :r] * np.sqrt(s[:r])
        GJ = Vt[:r].T * np.sqrt(s[:r])
        _exp_basis = (g, FI, GJ)
    return _exp_basis


def _prep_core_inputs(inputs, core):
    b, ch = core // 4, core % 4
    i0 = ch * CH
    x = np.ascontiguousarray(inputs["x"][b], np.float32)  # [N, D]
    pdir = np.ascontiguousarray(inputs["principal_dir"][b], np.float32)
    nrm = np.ascontiguousarray(inputs["normals"][b], np.float32)
    crv = inputs["curvature"][b].astype(np.float32)
    dens = inputs["density"][b].astype(np.float32)
    lin = inputs["linearity"][b].astype(np.float32)
    qkv_w = inputs["qkv_w"].astype(np.float32)
    qkv_b = inputs["qkv_b"].astype(np.float32)
    out_w = inputs["out_w"].astype(np.float32)

    xyz = x[:, :3]
    n2 = (xyz.astype(np.float64) ** 2).sum(-1).astype(np.float32)
    cr = np.cross(pdir, nrm)
    side = cr / (np.linalg.norm(cr, axis=-1, keepdims=True) + 1e-8)
    rowdot = (xyz * pdir).sum(-1)
    rowsidedot = (xyz * side).sum(-1)

    xhi, xlo = _split_hi_lo(xyz)
    n2hi, n2lo = _split_hi_lo(n2)
    shi, slo = _split_hi_lo(side)
    rdhi, rdlo = _split_hi_lo(rowdot)
    rshi, rslo = _split_hi_lo(rowsidedot)

    ci = crv[i0 : i0 + CH]
    di = dens[i0 : i0 + CH]
    li = lin[i0 : i0 + CH]
    s_i = np.sqrt(1.0 - 0.25 * (1.0 - li)).astype(np.float32)

    grid, FI, GJ = _get_exp_basis()
    gj = np.stack([np.interp(crv, grid, GJ[:, k]) for k in range(R_EXP)])
    fi = np.stack([np.interp(ci, grid, FI[:, k]) for k in range(R_EXP)])
    gfac = (GAMMA / FOLD) * dens  # j-side factor

    # keys are rotated so this core's queries sit at columns 0:CH
    perm = (np.arange(N) + i0) % N

    # GEO rows (fp16): 0-2 xhi_j, 3-5 xlo_j, 6 n2hi, 7 n2lo, 8 ones,
    # 13-15 xhi dup, 16 ones dup, 17.. expansion g-side
    geo = np.zeros((128, N), np.float32)
    geo[0:3] = xhi.T
    geo[3:6] = xlo.T
    geo[6] = n2hi
    geo[7] = n2lo
    geo[8] = 1.0
    geo[13:16] = xhi.T
    geo[16] = 1.0
    for k in range(R_EXP):
        geo[17 + 3 * k : 20 + 3 * k] = (gj[k] * gfac)[None, :] * nrm.T
    geo = geo[:, perm]

    # rhs m0 (f32): dp' = s_i * (rowdot_i - x_j . pdir_i)
    phi, plo = _split_hi_lo(pdir[i0 : i0 + CH])
    rhsm0 = np.zeros((128, CH), np.float32)
    rhsm0[0:3] = -phi.T * s_i
    rhsm0[3:6] = -phi.T * s_i
    rhsm0[13:16] = -plo.T * s_i
    rhsm0[8] = rdhi[i0 : i0 + CH] * s_i
    rhsm0[16] = rdlo[i0 : i0 + CH] * s_i

    rhsf = np.zeros((128, 6, CH), np.float32)
    # m-slot 0: -d2 (exact negation)
    xhic = xhi[i0 : i0 + CH]
    xloc = xlo[i0 : i0 + CH]
    rhsf[0:3, 0] = 2.0 * xhic.T
    rhsf[3:6, 0] = 2.0 * xhic.T
    rhsf[13:16, 0] = 2.0 * xloc.T
    rhsf[6, 0] = -1.0
    rhsf[7, 0] = -1.0
    rhsf[8, 0] = -n2hi[i0 : i0 + CH]
    rhsf[16, 0] = -n2lo[i0 : i0 + CH]
    # m-slot 1: expansion f-side
    nic = nrm[i0 : i0 + CH]
    for k in range(R_EXP):
        rhsf[17 + 3 * k : 20 + 3 * k, 1] = fi[k][None, :] * nic.T
    # m-slot 3 row 0: E_i for the gate
    rhsf[0, 3] = HALF_W * (0.5 + di)
    # m-slots 4/5: dp' rhs as fp16 hi + lo
    m0hi = rhsm0.astype(np.float16).astype(np.float32)
    rhsf[:, 4] = m0hi
    rhsf[:, 5] = rhsm0 - m0hi
    # m-slot 2: lateral = rowsidedot_i - x_j . side_i
    sh, sl = shi[i0 : i0 + CH], slo[i0 : i0 + CH]
    rhsf[0:3, 2] = -sh.T
    rhsf[3:6, 2] = -sh.T
    rhsf[13:16, 2] = -sl.T
    rhsf[8, 2] = rshi[i0 : i0 + CH]
    rhsf[16, 2] = rslo[i0 : i0 + CH]
    xT = np.ascontiguousarray(x.T)[:, perm]
    f16 = np.float16

    blob16 = np.zeros((128, B16C), f16)
    blob16[:, C_I50 : C_I50 + D] = (FOLD * np.eye(D, dtype=np.float32)).astype(
        f16
    )
    blob16[:, C_WV : C_WV + D] = qkv_w[:, 2 * D : 3 * D].astype(f16)
    wqh = qkv_w[:, 0:D].reshape(D, H, HD).astype(np.float64)
    wkh = qkv_w[:, D : 2 * D].reshape(D, H, HD).astype(np.float64)
    for h in range(H):
        m_h = SC * (wqh[:, h, :] @ wkh[:, h, :].T)  # [D, D]
        blob16[:, C_WK + h * D : C_WK + (h + 1) * D] = m_h.astype(f16)
    wo_a = out_w.reshape(H, HD, D).transpose(1, 0, 2)
    blob16[0:HD, C_WO : C_WO + H * D] = wo_a.reshape(HD, H * D).astype(f16)
    blob16[0:1, C_ONES : C_ONES + D] = 1.0
    # E4 block-ones for the finish bc broadcast
    for r in range(H):
        blob16[r, C_E4 + r * HD : C_E4 + (r + 1) * HD] = 1.0

    blob32 = np.zeros((128, B32C), np.float32)
    blob32[0:HD, 512:516] = (qkv_b[0:D] * SC).reshape(H, HD).T
    blob32[0:HD, 516:520] = qkv_b[D : 2 * D].reshape(H, HD).T
    blob32[0:HD, 520:524] = qkv_b[2 * D : 3 * D].reshape(H, HD).T
    blob32[0:1, 524 : 524 + D] = inputs["out_b"].astype(np.float32)[None, :]

    return {
        "blob16": blob16,
        "blob32": blob32,
        "xt": xT.astype(f16),
        "geo": geo.astype(f16),
        "rhs": rhsf.astype(f16),
    }


def _run(inputs, trace=False):
    has_bk = bool(np.any(inputs["qkv_b"][D : 2 * D]))
    has_bv = bool(np.any(inputs["qkv_b"][2 * D : 3 * D]))
    key = ("nc", has_bk, has_bv)
    if key not in _cache:
        _cache[key] = _build_program(has_bk, has_bv)
    nc = _cache[key]
    in_maps = [_prep_core_inputs(inputs, c) for c in range(NCORES)]
    res = run_bass_kernel_spmd(nc, in_maps, core_ids=list(range(NCORES)), trace=trace)
    full = np.empty((B, N, D), np.float32)
    for c in range(NCORES):
        b, ch = c // 4, c % 4
        o = res.results[c]["out"]  # [128, NSUB, D]
        full[b, ch * CH : (ch + 1) * CH, :] = o.transpose(1, 0, 2).reshape(
            CH, D
        )
    return full, res


def kernel(**inputs):
    out, _ = _run(inputs)
    return out


# revision 69
# speedup vs baseline: 1.0054x; 1.0054x over previous
"""MultiHeadGeometryAttention Trainium2 kernel (v3).

Sharding: 8 cores = (B=2) x (N=2048 split into 4 query chunks of 512).
Each core computes the NxN geometry bias + side gate once for its 512
queries (shared by all 4 heads), then all-head attention in transposed
layout S^T[j, i] so the PV matmul contracts over keys on partitions.

Key structure (156 us baseline -> this kernel):
  - exp(-|crv_i - crv_j|) * normal_sim folded into the geometry matmul
    as a rank-64 separable expansion (PE contraction rows are free).
  - aniso term = (A2'/50) dp^2 - d2 with sqrt(A2'/50) folded into the
    d_par rhs; bias accumulates in ONE PSUM bank (ACT Square writes
    dp'^2, the -d2 / expansion matmuls accumulate on top); one eviction
    per tile, folded into scores via a 50*I fp16 identity matmul.
  - Gate sigmoid computed as (1 + tanh(-8*(|lat|-E)))/2 so every ACT
    function (Exp/Square/Abs/Tanh/Copy) lives in ONE table set -> no
    table reloads -> geometry and attention fully interleave per key
    tile inside 8 PSUM banks: geometry pa+pb (pa reused for lat),
    score half-groups 2x2, PV accumulators 2 (4 heads at partition
    bases 0/64).
  - Scores: one batched scalar_tensor_tensor per 2-head half (gate
    broadcast over heads), one batched exp per (jt, 4 heads) with bias
    -2 so e^s stays in fp16 range (cancels in softmax), PV pipelined
    one jt behind so PE never waits on the exp.
  - All tensors ship fp16 (hi/lo split hi parts fp16-exact) in few
    packed DMA blobs; fp16 matmuls run at full PE rate.
"""

import math

import numpy as np

import concourse.bass as bass  # noqa: F401
import concourse.mybir as mybir
import concourse.tile as tile
from concourse import bacc
from concourse.bass import _add_dep_helper
from concourse.bass_utils import run_bass_kernel_spmd

# problem constants (fixed by the nn.Module config)
ALPHA0 = 1.0
BETA0 = 4.0
GAMMA = 0.5
SIGMA = 0.2
W_MIN, W_MAX = 0.05, 0.3
B, N, D, H = 2, 2048, 128, 4
HD = D // H  # 32
CH = 512  # query rows per core
NCORES = 8
NJT = N // 128  # 16 key tiles
NSUB = CH // 128  # 4 query subtiles

HALF_W = 0.5 * (W_MIN + W_MAX)  # 0.175
GATE_INV_SCALE = 1.0 / (0.25 * (W_MAX - W_MIN))  # 16.0
SC = 1.0 / math.sqrt(HD)
FOLD = 50.0  # geom tile carries bias/FOLD; folded back via FOLD*I matmul

R_EXP = 37  # rank of the exp(-|ci-cj|) separable expansion (geo rows 17..127)

LAG = 4  # attention trails geometry by this many key tiles
# geometry eviction engine per jt: True -> ACT, False -> DVE
GEOM_ACT = [False] * 16

# blob16 column offsets (keys are rotated per-core so xtc = xt[:, 0:CH])
C_I50, C_WV, C_WK, C_WO, C_ONES, C_E4 = 0, 128, 256, 768, 1280, 1408
B16C = 1536
# blob32: rhsm0 0-512, bq 512-516, bk 516-520, bv 520-524, ob row0 524-652
B32C = 652

F32 = mybir.dt.float32
F32R = mybir.dt.float32r
F16 = mybir.dt.float16
AF = mybir.ActivationFunctionType
ALU = mybir.AluOpType

_cache = {}


def _build_program(has_bk=False, has_bv=False):
    nc = bacc.Bacc(None)

    blob16 = nc.dram_tensor("blob16", [128, B16C], F16, kind="ExternalInput")
    blob32 = nc.dram_tensor("blob32", [128, B32C], F32, kind="ExternalInput")
    xt = nc.dram_tensor("xt", [D, N], F16, kind="ExternalInput")
    geo = nc.dram_tensor("geo", [128, N], F16, kind="ExternalInput")
    rhs = nc.dram_tensor("rhs", [128, 6, CH], F16, kind="ExternalInput")
    out = nc.dram_tensor("out", [128, NSUB, D], F32, kind="ExternalOutput")

    with tile.TileContext(nc) as tc, nc.allow_low_precision(
        reason="fp16 operands and f32r rounding are intentional"
    ):
        with (
            tc.tile_pool(name="const", bufs=1) as const,
            tc.tile_pool(name="tmp", bufs=1) as tmp,
            tc.tile_pool(name="s2p", bufs=4) as s2p,
            tc.tile_pool(name="ep", bufs=4) as ep,
            tc.tile_pool(name="gprep", bufs=2) as gprep,
            tc.tile_pool(name="latp", bufs=2) as latp,
        ):
            geo_sb = const.tile([128, N], F16, tag="geo", name="geo_sb")
            rhs_sb = const.tile([128, 6, CH], F16, tag="rhs", name="rhs_sb")
            b16 = const.tile([128, B16C], F16, tag="b16", name="b16")
            xt_sb = const.tile([D, N], F16, tag="xt", name="xt_sb")
            b32 = const.tile([128, B32C], F32, tag="b32", name="b32")
            nc.sync.dma_start(geo_sb[:, 0 : N // 4], geo[:, 0 : N // 4])
            nc.sync.dma_start(rhs_sb[:, 4:6, :], rhs[:, 4:6, :])
            nc.sync.dma_start(rhs_sb[:, 0:4, :], rhs[:, 0:4, :])
            nc.sync.dma_start(
                geo_sb[:, N // 4 : N // 2], geo[:, N // 4 : N // 2]
            )
            nc.sync.dma_start(geo_sb[:, N // 2 : N], geo[:, N // 2 : N])
            nc.sync.dma_start(b16, blob16[...])
            nc.sync.dma_start(xt_sb[:, 0 : N // 2], xt[:, 0 : N // 2])
            nc.sync.dma_start(xt_sb[:, N // 2 : N], xt[:, N // 2 : N])
            nc.sync.dma_start(b32, blob32[...])

            xtc_sb = xt_sb[:, 0:CH]
            i50_sb = b16[:, C_I50 : C_I50 + D]
            wv_sb = b16[:, C_WV : C_WV + D]
            ones_sb = b16[0:1, C_ONES : C_ONES + D]

            ob_sb = const.tile([1, D], F16, tag="ob", name="ob_sb")
            nc.gpsimd.tensor_copy(ob_sb, b32[0:1, 524 : 524 + D])

            nb2 = const.tile([128, 1], F32, tag="nb2", name="nb2")
            nc.gpsimd.memset(nb2, -2.0)

            qk4 = const.tile([D, H, CH], F16, tag="qk4", name="qk4")
            V = const.tile([128, NJT, H, HD + 1], F16, tag="V", name="V")
            E16 = const.tile([128, CH], F16, tag="E16", name="E16")
            geom16 = const.tile([128, NJT, CH], F16, tag="geom16", name="geom16")
            gate16 = const.tile([128, NJT, CH], F16, tag="gate16", name="gate16")
            headcat = const.tile([HD, H, CH], F16, tag="headcat", name="headcat")

            nc.gpsimd.memset(V[:, :, :, HD : HD + 1], 1.0)

            # ---- main loop: geometry and attention interleaved ----
            with (
                tc.tile_pool(name="a_pv", bufs=1, space="PSUM") as a_pv,
            ):
                pv0 = a_pv.tile([128, CH], F32, tag="pv0", name="pv0")
                pv1 = a_pv.tile([128, CH], F32, tag="pv1", name="pv1")
                pvb = [pv0, pv1]
                prev = [None]

                with (
                    tc.tile_pool(name="g_pa", bufs=1, space="PSUM") as g_pa,
                    tc.tile_pool(name="g_pb", bufs=1, space="PSUM") as g_pb,
                    tc.tile_pool(name="a_sg", bufs=2, space="PSUM") as a_sg,
                ):
                    gpre = [None]

                    # qk4_h = M_h^T @ x_chunk with M_h = SC * Wq_h Wk_h^T
                    # host-folded (kills the q->qT->k chain on device)
                    for h in range(2):
                        sgk = a_sg.tile([128, 2, CH], F32, tag="sg",
                                        name="sgk")
                        for hh in range(2):
                            nc.tensor.matmul(
                                sgk[:, hh, :],
                                lhsT=b16[:, C_WK + (2 * h + hh) * D
                                         : C_WK + (2 * h + hh + 1) * D],
                                rhs=xtc_sb,
                                start=True,
                                stop=True,
                            )
                        nc.scalar.copy(qk4[:, 2 * h : 2 * h + 2, :], sgk)
                    sge = a_sg.tile([128, 2, CH], F32, tag="sg", name="sge")
                    nc.tensor.matmul(
                        sge[:, 0, :], lhsT=ones_sb, rhs=rhs_sb[0:1, 3, :],
                        start=True, stop=True,
                    )
                    nc.scalar.copy(E16, sge[:, 0, :])

                    def emit_g(jt):
                        j0 = jt * 128
                        lhs = geo_sb[:, j0 : j0 + 128]
                        pa1 = g_pa.tile([128, CH], F32, tag="pa", name="pa1")
                        nc.tensor.matmul(pa1, lhsT=lhs, rhs=rhs_sb[:, 4, :],
                                         start=True, stop=False)
                        nc.tensor.matmul(pa1, lhsT=lhs, rhs=rhs_sb[:, 5, :],
                                         start=False, stop=True,
                                         skip_group_check=True)
                        pb = g_pb.tile([128, CH], F32, tag="pb", name="pb")
                        sq = nc.scalar.activation(pb, pa1, AF.Square)
                        pa2 = g_pa.tile([128, CH], F32, tag="pa", name="pa2")
                        nc.tensor.matmul(pa2, lhsT=lhs, rhs=rhs_sb[:, 2, :],
                                         start=True, stop=True)
                        m1 = nc.tensor.matmul(
                            pb, lhsT=lhs, rhs=rhs_sb[:, 0, :], start=False,
                            stop=False, skip_group_check=True,
                        )
                        # explicit RAW sync: the accumulate must not issue
                        # before the Square's PSUM write lands
                        _add_dep_helper(m1.ins, sq.ins, sync=True,
                                        reason="square-accum RAW")
                        nc.tensor.matmul(
                            pb, lhsT=lhs, rhs=rhs_sb[:, 1, :], start=False,
                            stop=True, skip_group_check=True,
                        )
                        if GEOM_ACT[jt]:
                            nc.scalar.copy(geom16[:, jt, :], pb)
                        else:
                            nc.vector.tensor_copy(geom16[:, jt, :], pb)
                        lata = latp.tile([128, CH], F16, tag="lata",
                                         name="lata")
                        nc.scalar.activation(lata, pa2, AF.Abs)
                        if jt % 2 == 0:
                            gpre[0] = gprep.tile([128, 2, CH], F16, tag="gp",
                                                 name="gpre")
                        nc.vector.tensor_tensor(
                            gpre[0][:, jt % 2, :], lata, E16, ALU.subtract
                        )
                        if jt % 2 == 1:
                            g = jt // 2
                            tgrp = latp.tile([128, 2, CH], F16, tag="tgrp",
                                             name="tgrp")
                            nc.scalar.activation(
                                tgrp, gpre[0], AF.Tanh,
                                scale=-0.5 * GATE_INV_SCALE,
                            )
                            # gate = 0.5 * (tanh + 1)
                            nc.vector.tensor_scalar(
                                gate16[:, g * 2 : g * 2 + 2, :], tgrp, 1.0,
                                0.5, ALU.add, ALU.mult,
                            )

                    def emit_a(jt):
                        s2 = s2p.tile([128, H, CH], F16, tag="s2", name="s2")
                        e = ep.tile([128, H, CH], F16, tag="e", name="e")
                        for half in range(2):
                            sg = a_sg.tile([128, 2, CH], F32, tag="sg",
                                           name="sg")
                            for hh in range(2):
                                h = 2 * half + hh
                                nc.tensor.matmul(
                                    sg[:, hh, :],
                                    lhsT=xt_sb[:, jt * 128 : (jt + 1) * 128],
                                    rhs=qk4[:, h, :],
                                    start=True,
                                    stop=False,
                                )
                                nc.tensor.matmul(
                                    sg[:, hh, :],
                                    lhsT=i50_sb,
                                    rhs=geom16[:, jt, :],
                                    start=False,
                                    stop=True,
                                    skip_group_check=True,
                                )
                            nc.vector.scalar_tensor_tensor(
                                s2[:, 2 * half : 2 * half + 2, :],
                                sg,
                                0.0,
                                gate16[:, jt : jt + 1, :].to_broadcast(
                                    [128, 2, CH]
                                ),
                                ALU.bypass,
                                ALU.mult,
                            )
                            nc.scalar.activation(
                                e[:, 2 * half : 2 * half + 2, :],
                                s2[:, 2 * half : 2 * half + 2, :],
                                AF.Exp, bias=nb2,
                            )
                        if prev[0] is not None:
                            emit_pv(*prev[0])
                        prev[0] = (jt, e)

                    def emit_pv(jt, e):
                        for h in range(H):
                            b0 = (h % 2) * 64
                            nc.tensor.matmul(
                                pvb[h // 2][b0 : b0 + HD + 1, :],
                                lhsT=V[:, jt, h, :],
                                rhs=e[:, h, :],
                                start=(jt == 0),
                                stop=(jt == NJT - 1),
                                skip_group_check=True,
                            )

                    def emit_v(vj):
                        sg = a_sg.tile([128, 2, CH], F32, tag="sg", name="sgv")
                        nc.tensor.matmul(
                            sg[:, 0, 0:D],
                            lhsT=xt_sb[:, vj * 128 : (vj + 1) * 128],
                            rhs=wv_sb,
                            start=True,
                            stop=True,
                        )
                        nc.vector.tensor_copy(
                            out=V[:, vj, :, 0:HD],
                            in_=sg[:, 0, 0:D].rearrange("p (h d) -> p h d",
                                                        h=H),
                        )

                    vper = (NJT + LAG - 1) // LAG  # V-projections per G step
                    for jt in range(NJT):
                        emit_g(jt)
                        if jt < LAG:
                            for vj in range(jt * vper,
                                            min((jt + 1) * vper, NJT)):
                                emit_v(vj)
                        else:
                            emit_a(jt - LAG)
                    for jt in range(NJT - LAG, NJT):
                        emit_a(jt)
                    emit_pv(*prev[0])

                # ---- finish: normalize, concat heads, project ----
                with tc.tile_pool(name="f_ps", bufs=2, space="PSUM") as f_ps:
                    recips = []
                    for h in range(H):
                        b0 = (h % 2) * 64
                        recip = tmp.tile([1, CH], F16, tag="recip",
                                         name="recip", bufs=4)
                        nc.vector.reciprocal(
                            recip, pvb[h // 2][b0 + HD : b0 + HD + 1, :]
                        )
                        recips.append(recip)
                    bc16s = []
                    for pair in range(2):
                        bc_ps = f_ps.tile([2 * HD, CH], F32, tag="bc",
                                          name="bc_ps")
                        for hh in range(2):
                            nc.tensor.matmul(
                                bc_ps[hh * HD : (hh + 1) * HD, :],
                                lhsT=ones_sb[0:1, 0:HD],
                                rhs=recips[2 * pair + hh],
                                start=True,
                                stop=True,
                            )
                        bc16 = tmp.tile([2 * HD, CH], F16, tag="bc16",
                                        name="bc16", bufs=2)
                        nc.scalar.copy(bc16, bc_ps)
                        bc16s.append(bc16)
                    for h in range(H):
                        b0 = (h % 2) * 64
                        nc.vector.scalar_tensor_tensor(
                            headcat[:, h, :],
                            pvb[h // 2][b0 : b0 + HD, :],
                            0.0,
                            bc16s[h // 2][(h % 2) * HD : (h % 2 + 1) * HD, :],
                            ALU.bypass,
                            ALU.mult,
                        )
                        if has_bv:
                            nc.scalar.activation(
                                headcat[:, h, :], headcat[:, h, :],
                                AF.Identity,
                                bias=b32[0:HD, 520 + h : 521 + h],
                            )

                    f_all = tmp.tile([128, NSUB, D], F32, tag="fall",
                                     name="f_all")
                    for s in range(NSUB):
                        fps = f_ps.tile([128, D], F32, tag="f", name="fps")
                        for h in range(H):
                            nc.tensor.matmul(
                                fps,
                                lhsT=headcat[:, h, s * 128 : (s + 1) * 128],
                                rhs=b16[0:HD, C_WO + h * D : C_WO + (h + 1) * D],
                                start=(h == 0),
                                stop=False,
                            )
                        nc.tensor.matmul(
                            fps, lhsT=ones_sb, rhs=ob_sb, start=False,
                            stop=True,
                        )
                        if s % 2:
                            nc.scalar.copy(f_all[:, s, :], fps)
                        else:
                            nc.vector.tensor_copy(f_all[:, s, :], fps)
                    nc.sync.dma_start(out[...], f_all)

    nc.finalize()
    return nc


def _split_hi_lo(v):
    """Split fp32 array into an fp16-exact hi part and the fp32 residual."""
    v = np.asarray(v, np.float32)
    hi = v.astype(np.float16).astype(np.float32)
    lo = (v.astype(np.float64) - hi).astype(np.float32)
    return hi, lo


_exp_basis = None


def _get_exp_basis():
    """Separable rank-R_EXP expansion of exp(-|a-b|) on [0,1]^2."""
    global _exp_basis
    if _exp_basis is None:
        g = np.linspace(0.0, 1.0, 2048)
        K = np.exp(-np.abs(g[:, None] - g[None, :]))
        U, s, Vt = np.linalg.svd(K)
        r = R_EXP
        FI = U[:, :r] * np.sqrt(s[:r])
        GJ = Vt[:r].T * np.sqrt(s[:r])
        _exp_basis = (g, FI, GJ)
    return _exp_basis


def _prep_core_inputs(inputs, core):
    b, ch = core // 4, core % 4
    i0 = ch * CH
    x = np.ascontiguousarray(inputs["x"][b], np.float32)  # [N, D]
    pdir = np.ascontiguousarray(inputs["principal_dir"][b], np.float32)
    nrm = np.ascontiguousarray(inputs["normals"][b], np.float32)
    crv = inputs["curvature"][b].astype(np.float32)
    dens = inputs["density"][b].astype(np.float32)
    lin = inputs["linearity"][b].astype(np.float32)
    qkv_w = inputs["qkv_w"].astype(np.float32)
    qkv_b = inputs["qkv_b"].astype(np.float32)
    out_w = inputs["out_w"].astype(np.float32)

    xyz = x[:, :3]
    n2 = (xyz.astype(np.float64) ** 2).sum(-1).astype(np.float32)
    cr = np.cross(pdir, nrm)
    side = cr / (np.linalg.norm(cr, axis=-1, keepdims=True) + 1e-8)
    rowdot = (xyz * pdir).sum(-1)
    rowsidedot = (xyz * side).sum(-1)

    xhi, xlo = _split_hi_lo(xyz)
    n2hi, n2lo = _split_hi_lo(n2)
    shi, slo = _split_hi_lo(side)
    rdhi, rdlo = _split_hi_lo(rowdot)
    rshi, rslo = _split_hi_lo(rowsidedot)

    ci = crv[i0 : i0 + CH]
    di = dens[i0 : i0 + CH]
    li = lin[i0 : i0 + CH]
    s_i = np.sqrt(1.0 - 0.25 * (1.0 - li)).astype(np.float32)

    grid, FI, GJ = _get_exp_basis()
    gj = np.stack([np.interp(crv, grid, GJ[:, k]) for k in range(R_EXP)])
    fi = np.stack([np.interp(ci, grid, FI[:, k]) for k in range(R_EXP)])
    gfac = (GAMMA / FOLD) * dens  # j-side factor

    # keys are rotated so this core's queries sit at columns 0:CH
    perm = (np.arange(N) + i0) % N

    # GEO rows (fp16): 0-2 xhi_j, 3-5 xlo_j, 6 n2hi, 7 n2lo, 8 ones,
    # 13-15 xhi dup, 16 ones dup, 17.. expansion g-side
    geo = np.zeros((128, N), np.float32)
    geo[0:3] = xhi.T
    geo[3:6] = xlo.T
    geo[6] = n2hi
    geo[7] = n2lo
    geo[8] = 1.0
    geo[13:16] = xhi.T
    geo[16] = 1.0
    for k in range(R_EXP):
        geo[17 + 3 * k : 20 + 3 * k] = (gj[k] * gfac)[None, :] * nrm.T
    geo = geo[:, perm]

    # rhs m0 (f32): dp' = s_i * (rowdot_i - x_j . pdir_i)
    phi, plo = _split_hi_lo(pdir[i0 : i0 + CH])
    rhsm0 = np.zeros((128, CH), np.float32)
    rhsm0[0:3] = -phi.T * s_i
    rhsm0[3:6] = -phi.T * s_i
    rhsm0[13:16] = -plo.T * s_i
    rhsm0[8] = rdhi[i0 : i0 + CH] * s_i
    rhsm0[16] = rdlo[i0 : i0 + CH] * s_i

    rhsf = np.zeros((128, 6, CH), np.float32)
    # m-slot 0: -d2 (exact negation)
    xhic = xhi[i0 : i0 + CH]
    xloc = xlo[i0 : i0 + CH]
    rhsf[0:3, 0] = 2.0 * xhic.T
    rhsf[3:6, 0] = 2.0 * xhic.T
    rhsf[13:16, 0] = 2.0 * xloc.T
    rhsf[6, 0] = -1.0
    rhsf[7, 0] = -1.0
    rhsf[8, 0] = -n2hi[i0 : i0 + CH]
    rhsf[16, 0] = -n2lo[i0 : i0 + CH]
    # m-slot 1: expansion f-side
    nic = nrm[i0 : i0 + CH]
    for k in range(R_EXP):
        rhsf[17 + 3 * k : 20 + 3 * k, 1] = fi[k][None, :] * nic.T
    # m-slot 3 row 0: E_i for the gate
    rhsf[0, 3] = HALF_W * (0.5 + di)
    # m-slots 4/5: dp' rhs as fp16 hi + lo
    m0hi = rhsm0.astype(np.float16).astype(np.float32)
    rhsf[:, 4] = m0hi
    rhsf[:, 5] = rhsm0 - m0hi
    # m-slot 2: lateral = rowsidedot_i - x_j . side_i
    sh, sl = shi[i0 : i0 + CH], slo[i0 : i0 + CH]
    rhsf[0:3, 2] = -sh.T
    rhsf[3:6, 2] = -sh.T
    rhsf[13:16, 2] = -sl.T
    rhsf[8, 2] = rshi[i0 : i0 + CH]
    rhsf[16, 2] = rslo[i0 : i0 + CH]
    xT = np.ascontiguousarray(x.T)[:, perm]
    f16 = np.float16

    blob16 = np.zeros((128, B16C), f16)
    blob16[:, C_I50 : C_I50 + D] = (FOLD * np.eye(D, dtype=np.float32)).astype(
        f16
    )
    blob16[:, C_WV : C_WV + D] = qkv_w[:, 2 * D : 3 * D].astype(f16)
    wqh = qkv_w[:, 0:D].reshape(D, H, HD).astype(np.float64)
    wkh = qkv_w[:, D : 2 * D].reshape(D, H, HD).astype(np.float64)
    for h in range(H):
        m_h = SC * (wqh[:, h, :] @ wkh[:, h, :].T)  # [D, D]
        blob16[:, C_WK + h * D : C_WK + (h + 1) * D] = m_h.astype(f16)
    wo_a = out_w.reshape(H, HD, D).transpose(1, 0, 2)
    blob16[0:HD, C_WO : C_WO + H * D] = wo_a.reshape(HD, H * D).astype(f16)
    blob16[0:1, C_ONES : C_ONES + D] = 1.0
    # E4 block-ones for the finish bc broadcast
    for r in range(H):
        blob16[r, C_E4 + r * HD : C_E4 + (r + 1) * HD] = 1.0

    blob32 = np.zeros((128, B32C), np.float32)
    blob32[0:HD, 512:516] = (qkv_b[0:D] * SC).reshape(H, HD).T
    blob32[0:HD, 516:520] = qkv_b[D : 2 * D].reshape(H, HD).T
    blob32[0:HD, 520:524] = qkv_b[2 * D : 3 * D].reshape(H, HD).T
    blob32[0:1, 524 : 524 + D] = inputs["out_b"].astype(np.float32)[None, :]

    return {
        "blob16": blob16,
        "blob32": blob32,
        "xt": xT.astype(f16),
        "geo": geo.astype(f16),
        "rhs": rhsf.astype(f16),
    }


def _run(inputs, trace=False):
    has_bk = bool(np.any(inputs["qkv_b"][D : 2 * D]))
    has_bv = bool(np.any(inputs["qkv_b"][2 * D : 3 * D]))
    key = ("nc", has_bk, has_bv)
    if key not in _cache:
        _cache[key] = _build_program(has_bk, has_bv)
    nc = _cache[key]
    in_maps = [_prep_core_inputs(inputs, c) for c in range(NCORES)]
    res = run_bass_kernel_spmd(nc, in_maps, core_ids=list(range(NCORES)), trace=trace)
    full = np.empty((B, N, D), np.float32)
    for c in range(NCORES):
        b, ch = c // 4, c % 4
        o = res.results[c]["out"]  # [128, NSUB, D]
        full[b, ch * CH : (ch + 1) * CH, :] = o.transpose(1, 0, 2).reshape(
            CH, D
        )
    return full, res


def kernel(**inputs):
    out, _ = _run(inputs)
    return out
